# revision 1
# baseline (speedup 1.0000x reference)
"""v3: symmetry + fp8e4m3 DoubleRow GEMM for the denominator.

Same structure as kernel2 (rotation, 5 column groups, rowsum/colsum/pos
partials, host combine), but the similarity GEMM runs in fp8 with
perf_mode=DoubleRow: z is quantized to e4m3, bounced through DRAM as uint16
byte-pairs, xbar-transposed (u16), and contracted 256 d per matmul via the
3D [Ki, 2, N] pair AP. Positives stay on the bf16 path (precision), so only
the exp-sum denominators see fp8 noise, which averages out over 8191 terms.
"""

import numpy as np

try:
    import concourse.bass as bass
except ImportError:
    import sys

    for _p in ("/opt/trn_rl_repo", "/root/.axon_site/_ro/trn_rl_repo"):
        if _p not in sys.path:
            sys.path.append(_p)
    import concourse.bass as bass

import concourse.mybir as mybir
import concourse.tile as tile
from concourse import bacc
from concourse.bass_utils import run_bass_kernel_spmd

F32 = mybir.dt.float32
BF16 = mybir.dt.bfloat16
FP8 = mybir.dt.float8e4
U16 = mybir.dt.uint16
AF = mybir.ActivationFunctionType
ALU = mybir.AluOpType

B = 4096
D = 1024
R = 2 * B
N_CORES = 8
LOCAL = R // N_CORES
INV_TEMP = 2.0
E2 = float(np.exp(INV_TEMP))

NGC = 5
KK = D // 256      # 4 contraction chunks of 256 (DoubleRow pairs)
CS_G = (1, 2, 3)


def build_nc():
    nc = bacc.Bacc("TRN2", target_bir_lowering=False, debug=False)

    embr = nc.dram_tensor("embr", [NGC * 1024, D], F32, kind="ExternalInput")
    out_rowsum = nc.dram_tensor("rowsum", [128, 8], F32, kind="ExternalOutput")
    out_pos = nc.dram_tensor("pos", [128, 8], F32, kind="ExternalOutput")
    out_colsum = nc.dram_tensor("colsum", [1, 3072], F32, kind="ExternalOutput")

    with tile.TileContext(nc) as tc:
        with (
            tc.tile_pool(name="xin", bufs=6) as xin_pool,
            tc.tile_pool(name="zrow", bufs=4) as zrow_pool,
            tc.tile_pool(name="zkeep", bufs=8) as zkeep_pool,
            tc.tile_pool(name="stats", bufs=16) as stats_pool,
            tc.tile_pool(name="sqj", bufs=3) as sqj_pool,
            tc.tile_pool(name="zt", bufs=1) as zt_pool,
            tc.tile_pool(name="acc", bufs=1) as acc_pool,
            tc.tile_pool(name="ej", bufs=6) as ej_pool,
            tc.tile_pool(name="dram", bufs=1, space="DRAM") as dram_pool,
            tc.tile_pool(name="psum", bufs=6, space="PSUM") as psum_pool,
            tc.tile_pool(name="pscs", bufs=2, space="PSUM") as pscs_pool,
        ):
            # transposed fp8 z as u16 byte-pairs: ztu[kk][pair_p, row]
            # covers d = 256*kk + 2*p + i
            ztus = [
                zt_pool.tile([128, NGC * 1024], U16, tag=f"ztu{kk}", name=f"ztu{kk}")
                for kk in range(KK)
            ]
            zbufs = [
                dram_pool.tile([1024, D // 2], U16, tag=f"zb{g}", name=f"zb{g}")
                for g in range(NGC)
            ]

            pos = acc_pool.tile([128, 8], F32, name="pos")
            rs = acc_pool.tile([128, 8, 2 * NGC], F32, name="rs")
            colsum_acc = acc_pool.tile([1, 3072], F32, name="colsum_acc")
            ones = acc_pool.tile([128, 1], BF16, name="ones")
            nc.vector.memset(ones[:], 1.0)

            keep = [None] * 8

            # ---- Phase 1: normalize, quantize to fp8, bounce, u16-transpose
            for g in range(NGC):
                for tl in range(8):
                    t = g * 8 + tl
                    x = xin_pool.tile([128, D], F32, tag="x", name=f"x{t}")
                    nc.sync.dma_start(out=x[:], in_=embr[t * 128 : (t + 1) * 128, :])

                    sqj = sqj_pool.tile([128, D], F32, tag="sqj", name=f"sqj{t}")
                    ssq = stats_pool.tile([128, 1], F32, tag="ssq", name=f"ssq{t}")
                    nc.scalar.activation(
                        out=sqj[:], in_=x[:], func=AF.Square, accum_out=ssq[:]
                    )
                    nrm = stats_pool.tile([128, 1], F32, tag="nrm", name=f"nrm{t}")
                    nc.scalar.sqrt(nrm[:], ssq[:])
                    inv = stats_pool.tile([128, 1], F32, tag="inv", name=f"inv{t}")
                    nc.vector.reciprocal(out=inv[:], in_=nrm[:])

                    # fp8 path (denominator GEMM)
                    z8 = zrow_pool.tile([128, D], FP8, tag="z8", name=f"z8_{t}")
                    nc.vector.tensor_scalar_mul(z8[:], x[:], inv[:])
                    nc.sync.dma_start(
                        out=zbufs[g][tl * 128 : (tl + 1) * 128, :],
                        in_=z8.bitcast(U16)[:],
                    )

                    # bf16 path (positives only)
                    if t < 8:
                        zb = zkeep_pool.tile([128, D], BF16, tag="zk", name=f"zk{t}")
                        keep[t] = zb
                        nc.vector.tensor_scalar_mul(zb[:], x[:], inv[:])
                    elif 32 <= t < 40:
                        zb = zrow_pool.tile([128, D], BF16, tag="zr", name=f"zr{t}")
                        nc.vector.tensor_scalar_mul(zb[:], x[:], inv[:])
                        tt = t - 32
                        pj = sqj_pool.tile([128, D], F32, tag="pj", name=f"pj{t}")
                        nc.vector.tensor_mul(pj[:], keep[tt][:], zb[:])
                        nc.vector.tensor_reduce(
                            out=pos[:, tt : tt + 1],
                            in_=pj[:],
                            axis=mybir.AxisListType.X,
                            op=ALU.add,
                        )

                for kk in range(KK):
                    nc.sync.dma_start(
                        out=ztus[kk][:, g * 1024 : (g + 1) * 1024],
                        in_=zbufs[g][:, kk * 128 : (kk + 1) * 128],
                        transpose=True,
                    )

            # deinterleaved fp8 pair tiles [128, 2, NGC*1024]: byte-interleaved
            # pair strides fail the LDWEIGHTS ISA check, so DVE-copy into the
            # tile_matmul-style layout (pair stride = NGC*1024 bytes).
            ztds = [
                zt_pool.tile([128, 2, NGC * 1024], FP8, tag=f"ztd{kk}", name=f"ztd{kk}")
                for kk in range(KK)
            ]
            for kk in range(KK):
                for g in range(NGC):
                    src = ztus[kk].bitcast(FP8)[
                        :, g * 2048 : (g + 1) * 2048
                    ].rearrange("p (n two) -> p two n", two=2)
                    nc.vector.tensor_copy(
                        ztds[kk][:, :, g * 1024 : (g + 1) * 1024], src
                    )
            z3s = [ztds[kk] for kk in range(KK)]

            # ---- Phase 2: fp8 DoubleRow GEMM + exp/rowsum (+ colsum for g 1..3)
            for g in range(NGC):
                want_cs = g in CS_G
                css = None
                if want_cs:
                    css = [
                        pscs_pool.tile([1, 512], F32, tag="cs", name=f"cs{g}_{cb}")
                        for cb in range(2)
                    ]
                for m in range(8):
                    pss = [
                        psum_pool.tile([128, 512], F32, tag="ps", name=f"ps{g}_{m}_{cb}")
                        for cb in range(2)
                    ]
                    for kk in range(KK):
                        lhsT = z3s[kk][:, :, m * 128 : (m + 1) * 128]
                        for cb in range(2):
                            nc.tensor.matmul(
                                pss[cb][:],
                                lhsT,
                                z3s[kk][
                                    :, :, g * 1024 + cb * 512 : g * 1024 + (cb + 1) * 512
                                ],
                                start=(kk == 0),
                                stop=(kk == KK - 1),
                                perf_mode=mybir.MatmulPerfMode.DoubleRow,
                            )
                    for cb in range(2):
                        ej = ej_pool.tile(
                            [128, 512], BF16, tag="ej", name=f"ej{g}_{m}_{cb}"
                        )
                        j = g * 2 + cb
                        nc.scalar.activation(
                            out=ej[:],
                            in_=pss[cb][:],
                            func=AF.Exp,
                            bias=0.0,
                            scale=INV_TEMP,
                            accum_out=rs[:, m, j : j + 1],
                        )
                        if want_cs:
                            nc.tensor.matmul(
                                css[cb][:],
                                ones[:],
                                ej[:],
                                start=(m == 0),
                                stop=(m == 7),
                            )
                if want_cs:
                    for cb in range(2):
                        off = (g - 1) * 1024 + cb * 512
                        nc.vector.tensor_copy(colsum_acc[:, off : off + 512], css[cb][:])

            # ---- Phase 3: write partial outputs
            rowsum = acc_pool.tile([128, 8], F32, name="rowsum")
            nc.vector.tensor_reduce(
                out=rowsum[:], in_=rs[:], axis=mybir.AxisListType.X, op=ALU.add
            )
            nc.sync.dma_start(out=out_rowsum[:, :], in_=rowsum[:])
            nc.sync.dma_start(out=out_pos[:, :], in_=pos[:])
            nc.sync.dma_start(out=out_colsum[:, :], in_=colsum_acc[:])

    nc.compile()
    return nc


_NC = None


def _get_nc():
    global _NC
    if _NC is None:
        _NC = build_nc()
    return _NC


def make_in_maps(emb_i, emb_j):
    reps = np.concatenate(
        [np.asarray(emb_i, np.float32), np.asarray(emb_j, np.float32)], axis=0
    )
    rolled = [np.roll(reps, -c * LOCAL, axis=0)[: NGC * 1024] for c in range(N_CORES)]
    return [{"embr": np.ascontiguousarray(r)} for r in rolled]


def run_spmd(in_maps, trace=False, **kwargs):
    return run_bass_kernel_spmd(
        _get_nc(), in_maps, core_ids=list(range(N_CORES)), trace=trace, **kwargs
    )


def combine(results):
    rowsum = np.stack(
        [r["rowsum"].astype(np.float64).T.reshape(LOCAL) for r in results]
    )
    pos = np.stack([r["pos"].astype(np.float64).T.reshape(LOCAL) for r in results])
    colsum = np.stack(
        [r["colsum"].astype(np.float64).reshape(3, 1024) for r in results]
    )
    denom = rowsum.copy()
    for b in range(N_CORES):
        for gp in CS_G:
            denom[b] += colsum[(b - gp) % N_CORES][gp - 1]
    denom -= E2
    loss_rows = np.log(denom) - INV_TEMP * pos
    return float(loss_rows.sum() / R)


def kernel(emb_i, emb_j):
    res = run_spmd(make_in_maps(emb_i, emb_j))
    return np.array(combine(res.results), dtype=np.float32)



# revision 15
# speedup vs baseline: 1.3108x; 1.3108x over previous
"""NT-Xent contrastive loss on 8 trn2 cores — inv-exchange architecture.

Each core receives the full rolled 5120-row window in bf16 (host-sharded)
but computes norms ONLY for its local 1024 rows (DVE square+reduce, ACT
Ln/Exp rsqrt with the fp8 pre-scale folded in). The tiny per-core inv
vectors are AllGathered (32KB — cheap) and gathered back per-window-row,
so the 5x-redundant norm work disappears. All 40 window tiles are scaled
to bf16 z on DVE (4x mode), cast to fp8 in-flight by SWDGE bounce DMAs,
and pulled back through dma_gather(transpose=True) directly into the u16
pair-interleaved transposed layout. The similarity GEMM runs
DoubleRowSwInterleave with natural interleaved weights (psum rows come
back reversed — host relabels) and byte-strided moving APs: no
deinterleave pass. exp+rowsum on ACT in [128,1024] psum tiles; colsums
via ones-matmuls for the 3 symmetric exchange groups; positives from the
group-4 psum anti-diagonal. Host: rowsums + shifted colsums - e^2 - 2*pos.
"""

import numpy as np
import ml_dtypes

try:
    import concourse.bass as bass
except ImportError:
    import sys

    for _p in ("/opt/trn_rl_repo", "/root/.axon_site/_ro/trn_rl_repo"):
        if _p not in sys.path:
            sys.path.append(_p)
    import concourse.bass as bass

import concourse.mybir as mybir
import concourse.tile as tile
from concourse import bacc
from concourse.bass_utils import run_bass_kernel_spmd

F32 = mybir.dt.float32
BF16 = mybir.dt.bfloat16
FP8 = mybir.dt.float8e4
U16 = mybir.dt.uint16
I16 = mybir.dt.int16
AF = mybir.ActivationFunctionType
ALU = mybir.AluOpType
MM = mybir.MatmulPerfMode

B = 4096
D = 1024
R = 2 * B
N_CORES = 8
LOCAL = R // N_CORES          # 1024 rows per core
NG = 5                        # column groups per core (symmetry)
W = NG * LOCAL                # 5120 window rows/columns
KK = 4                        # u16 pair chunks (128 partitions each)
CS_G = (1, 2, 3)              # groups recovered via colsum exchange
INV_TEMP = 2.0
E2 = float(np.exp(INV_TEMP))
S = 16.0                      # fp8 pre-scale (avoids e4m3 subnormals)
EXP_SCALE = INV_TEMP / (S * S)


def build_nc():
    nc = bacc.Bacc("TRN2", target_bir_lowering=False, debug=False)

    xwin = nc.dram_tensor("xwin", [W, D], BF16, kind="ExternalInput")
    # z8 gather indices (identity into z8win; static, same on every core):
    # chunk h=j//512, i=j%512 -> slot [i%16 (+16), g, h*32 + i//16]
    idxs = nc.dram_tensor("idxs", [128, NG, LOCAL // 16], I16, kind="ExternalInput")
    # inv block-gather indices (per-core): window 64-row block -> global block
    invidx = nc.dram_tensor("invidx", [128, 8], I16, kind="ExternalInput")
    aeye = nc.dram_tensor("aeye", [128, 128], F32, kind="ExternalInput")

    out_rowsum = nc.dram_tensor("rowsum", [128, 8], F32, kind="ExternalOutput")
    out_pos = nc.dram_tensor("pos", [128, 8], F32, kind="ExternalOutput")
    out_colsum = nc.dram_tensor("colsum", [1, 3072], F32, kind="ExternalOutput")

    invloc = nc.dram_tensor("invloc", [LOCAL], F32)
    invfull = nc.dram_tensor("invfull", [R], F32, addr_space="Shared")
    z8win = nc.dram_tensor("z8win", [W, D], FP8)

    with tile.TileContext(nc) as tc:
        with (
            tc.tile_pool(name="xin", bufs=1) as xin_pool,
            tc.tile_pool(name="zb", bufs=2) as zb_pool,
            tc.tile_pool(name="zt", bufs=1) as zt_pool,
            tc.tile_pool(name="stats", bufs=1) as stats_pool,
            tc.tile_pool(name="scr", bufs=3) as scr_pool,
            tc.tile_pool(name="ej", bufs=3) as ej_pool,
            tc.tile_pool(name="ejcs", bufs=12) as ejcs_pool,
            tc.tile_pool(name="psum", bufs=3, space="PSUM") as psum_pool,
            tc.tile_pool(name="pscs", bufs=2, space="PSUM") as pscs_pool,
        ):
            # ---------- static small inputs ----------
            idx_sb = stats_pool.tile([128, NG, LOCAL // 16], I16, name="idx_sb")
            nc.sync.dma_start(out=idx_sb[:], in_=idxs[:, :, :])
            iidx_sb = stats_pool.tile([128, 8], I16, name="iidx_sb")
            nc.sync.dma_start(out=iidx_sb[:], in_=invidx[:, :])
            ae = stats_pool.tile([128, 128], F32, name="ae")
            nc.sync.dma_start(out=ae[:], in_=aeye[:, :])
            ones = stats_pool.tile([128, 1], BF16, name="ones")
            nc.vector.memset(ones[:], 1.0)

            ssq = stats_pool.tile([128, 8], F32, name="ssq")
            inv = stats_pool.tile([128, 8], F32, name="inv")
            rs = stats_pool.tile([128, 8 * NG], F32, name="rs")
            pos = stats_pool.tile([128, 8], F32, name="pos")
            colsum_acc = stats_pool.tile([1, 3072], F32, name="colsum_acc")

            # ---------- phase N: window loads + LOCAL norms ----------
            xg = []  # 5 group tiles [128, 8, 1024]
            for g in range(NG):
                xt = xin_pool.tile([128, 8, D], BF16, name=f"xg{g}")
                xg.append(xt)
                nc.sync.dma_start(
                    out=xt[:],
                    in_=xwin[g * LOCAL : (g + 1) * LOCAL, :].rearrange(
                        "(t p) d -> p t d", p=128
                    ),
                )
            for t in range(8):
                scr = scr_pool.tile([128, D], BF16, tag="nscr", name=f"nscr{t}")
                nc.vector.tensor_mul(scr[:], xg[0][:, t, :], xg[0][:, t, :])
                nc.vector.tensor_reduce(
                    out=ssq[:, t : t + 1], in_=scr[:],
                    axis=mybir.AxisListType.X, op=ALU.add,
                )
            # inv = S / sqrt(ssq) = exp(-0.5*ln(ssq) + ln(S))
            lns = stats_pool.tile([128, 8], F32, name="lns")
            lnS = stats_pool.tile([128, 1], F32, name="lnS")
            nc.vector.memset(lnS[:], float(np.log(S)))
            nc.scalar.activation(out=lns[:], in_=ssq[:], func=AF.Ln)
            nc.scalar.activation(
                out=inv[:], in_=lns[:], func=AF.Exp, bias=lnS[:], scale=-0.5
            )

            # ---------- phase C: exchange inv ----------
            nc.sync.dma_start(
                out=invloc.rearrange("(t p) -> p t", p=128), in_=inv[:, :]
            )
            nc.gpsimd.collective_compute(
                "AllGather", ALU.bypass,
                ins=[invloc[:]], outs=[invfull[:]],
                replica_groups=[list(range(N_CORES))],
            )
            # block-gather inv (64 f32 = 256B blocks), then redistribute:
            # pre[b', :] = invfull[window block b'] for b' in 0..80
            pre = stats_pool.tile([128, 128], F32, name="pre")
            nc.vector.memset(pre[:], 1.0)
            nc.gpsimd.dma_gather(
                out_ap=pre[:, 0:64].unsqueeze(1),
                in_ap=invfull.rearrange("(r o) -> r o", o=64),
                idxs_ap=iidx_sb[:],
                num_idxs=80, num_idxs_reg=80,
                elem_size=64, transpose=False,
            )
            preT = stats_pool.tile([128, 128], F32, name="preT")
            nc.vector.transpose(preT[:], pre[:])
            # invw[p, w] = inv(window row w*128+p):
            #   p<64:  preT[p, 2w]      p>=64: preT[p-64, 2w+1]
            invw = stats_pool.tile([128, W // 128], F32, name="invw")
            nc.vector.tensor_copy(invw[0:64, :], preT[0:64, 0 : 2 * (W // 128) : 2])
            nc.vector.tensor_copy(invw[64:128, :], preT[0:64, 1 : 2 * (W // 128) : 2])

            # ---------- scale + fp8 cast (SWDGE) ----------
            for g in range(NG):
                zbg = zb_pool.tile([128, 8, D], BF16, tag="zbg", name=f"zbg{g}")
                for t in range(8):
                    w = g * 8 + t
                    sc = inv[:, t : t + 1] if g == 0 else invw[:, w : w + 1]
                    nc.vector.tensor_scalar_mul(zbg[:, t, :], xg[g][:, t, :], sc)
                for h in range(2):
                    nc.gpsimd.dma_start(
                        out=z8win[
                            g * LOCAL + h * 512 : g * LOCAL + (h + 1) * 512, :
                        ].rearrange("(t p) d -> p t d", p=128),
                        in_=zbg[:, h * 4 : (h + 1) * 4, :],
                    )

            # ---------- gather-transpose into pair-interleaved layout ----------
            ztus = {}
            for g in range(NG):
                for h in range(2):
                    ztu_gh = zt_pool.tile([128, KK, 512], U16, name=f"ztu{g}_{h}")
                    ztus[(g, h)] = ztu_gh
                    nc.gpsimd.dma_gather(
                        out_ap=ztu_gh[:],
                        in_ap=z8win.bitcast(U16)[:, :],
                        idxs_ap=idx_sb[:, g, h * 32 : (h + 1) * 32],
                        num_idxs=512, num_idxs_reg=512,
                        elem_size=D // 2, transpose=True,
                    )

            # ---------- phase G: GEMM + exp (+colsum, +pos) ----------
            # SwInterleave: psum partition p = local row m*128 + (127-p)
            cs_tiles = {}

            def emit_colsum(g):
                css = cs_tiles[g]
                for m in range(8):
                    ej = cs_tiles[(g, m)]
                    for cb in range(2):
                        nc.tensor.matmul(
                            css[cb][:], ones[:],
                            ej[:, cb * 512 : (cb + 1) * 512],
                            start=(m == 0), stop=(m == 7),
                        )
                for cb in range(2):
                    off = (g - 1) * 1024 + cb * 512
                    nc.vector.tensor_copy(colsum_acc[:, off : off + 512], css[cb][:])

            for g in range(NG):
                if g - 1 in CS_G:
                    emit_colsum(g - 1)
                if g in CS_G:
                    cs_tiles[g] = [
                        pscs_pool.tile([1, 512], F32, tag="cs", name=f"cs{g}_{cb}")
                        for cb in range(2)
                    ]
                for m in range(8):
                    ps = psum_pool.tile([128, 1024], F32, tag="ps", name=f"ps{g}_{m}")
                    for kk in range(KK):
                        w_ap = ztus[(0, m // 4)][
                            :, kk, (m % 4) * 128 : (m % 4 + 1) * 128
                        ].bitcast(FP8)
                        for cb in range(2):
                            rhs = (
                                ztus[(g, cb)][:, kk, :]
                                .bitcast(FP8)
                                .rearrange("p (n two) -> p two n", two=2)
                            )
                            nc.tensor.matmul(
                                ps[:, cb * 512 : (cb + 1) * 512],
                                w_ap, rhs,
                                start=(kk == 0), stop=(kk == KK - 1),
                                perf_mode=MM.DoubleRowSwInterleave,
                            )
                    if g in CS_G:
                        ej = ejcs_pool.tile(
                            [128, 1024], BF16, tag="ejcs", name=f"ej{g}_{m}"
                        )
                        cs_tiles[(g, m)] = ej
                    else:
                        ej = ej_pool.tile([128, 1024], BF16, tag="ej", name=f"ej{g}_{m}")
                    nc.scalar.activation(
                        out=ej[:], in_=ps[:], func=AF.Exp, bias=0.0, scale=EXP_SCALE,
                        accum_out=rs[:, (m * NG + g) : (m * NG + g) + 1],
                    )
                    if g == 4:
                        pscr = scr_pool.tile([128, 128], F32, tag="pscr", name=f"pscr{m}")
                        nc.vector.tensor_tensor(
                            out=pscr[:], in0=ps[:, m * 128 : (m + 1) * 128],
                            in1=ae[:], op=ALU.mult,
                        )
                        nc.vector.tensor_reduce(
                            out=pos[:, m : m + 1], in_=pscr[:],
                            axis=mybir.AxisListType.X, op=ALU.add,
                        )
            emit_colsum(NG - 2)

            # ---------- outputs ----------
            rowsum = stats_pool.tile([128, 8], F32, name="rowsum")
            nc.vector.tensor_reduce(
                out=rowsum[:],
                in_=rs[:, :].rearrange("p (m g) -> p m g", g=NG),
                axis=mybir.AxisListType.X, op=ALU.add,
            )
            nc.sync.dma_start(out=out_rowsum[:, :], in_=rowsum[:])
            nc.sync.dma_start(out=out_pos[:, :], in_=pos[:])
            nc.sync.dma_start(out=out_colsum[:, :], in_=colsum_acc[:])

    nc.compile()
    return nc


_NC = None


def _get_nc():
    global _NC
    if _NC is None:
        _NC = build_nc()
    return _NC


_Z8IDX = np.zeros((128, NG, LOCAL // 16), np.int16)
for _g in range(NG):
    for _j in range(LOCAL):
        _h, _i = divmod(_j, 512)
        _p, _c = _i % 16, _h * 32 + _i // 16
        _Z8IDX[_p, _g, _c] = _g * LOCAL + _j
        _Z8IDX[16 + _p, _g, _c] = _g * LOCAL + _j

_AEYE = np.eye(128, dtype=np.float32)[:, ::-1].copy()


def _make_invidx(core):
    idx = np.zeros((128, 8), np.int16)
    for b in range(W // 64):
        blk = (core * (LOCAL // 64) + b) % (R // 64)
        idx[b % 16, b // 16] = blk
        idx[16 + b % 16, b // 16] = blk
    return idx


def make_in_maps(emb_i, emb_j):
    reps = np.concatenate(
        [np.asarray(emb_i, np.float32), np.asarray(emb_j, np.float32)], axis=0
    ).astype(ml_dtypes.bfloat16)
    maps = []
    for c in range(N_CORES):
        win = np.roll(reps, -c * LOCAL, axis=0)[:W]
        maps.append({
            "xwin": np.ascontiguousarray(win),
            "idxs": _Z8IDX,
            "invidx": _make_invidx(c),
            "aeye": _AEYE,
        })
    return maps


def run_spmd(in_maps, trace=False, **kwargs):
    return run_bass_kernel_spmd(
        _get_nc(), in_maps, core_ids=list(range(N_CORES)), trace=trace, **kwargs
    )


def combine(results):
    # SwInterleave row reversal: psum partition p of m-block = row m*128+(127-p)
    rowsum = np.zeros((N_CORES, LOCAL))
    pos = np.zeros((N_CORES, LOCAL))
    rev = 127 - np.arange(128)
    for c in range(N_CORES):
        rsum = results[c]["rowsum"].astype(np.float64)  # [128 p, 8 m]
        ps = results[c]["pos"].astype(np.float64)
        for m in range(8):
            rowsum[c, m * 128 + rev] = rsum[:, m]
            pos[c, m * 128 + rev] = ps[:, m]
    colsum = np.stack(
        [r["colsum"].astype(np.float64).reshape(3, 1024) for r in results]
    )
    denom = rowsum.copy()
    for b in range(N_CORES):
        for gp in CS_G:
            denom[b] += colsum[(b - gp) % N_CORES][gp - 1]
    denom -= E2
    loss_rows = np.log(denom) - (INV_TEMP / (S * S)) * pos
    return float(loss_rows.sum() / R)


def kernel(emb_i, emb_j):
    res = run_spmd(make_in_maps(emb_i, emb_j))
    return np.array(combine(res.results), dtype=np.float32)


# revision 16
# speedup vs baseline: 1.4628x; 1.1160x over previous
"""NT-Xent contrastive loss on 8 trn2 cores — inv-exchange architecture.

Each core receives the full rolled 5120-row window in bf16 (host-sharded)
but computes norms ONLY for its local 1024 rows (DVE square+reduce, ACT
Ln/Exp rsqrt with the fp8 pre-scale folded in). The tiny per-core inv
vectors are AllGathered (32KB — cheap) and gathered back per-window-row,
so the 5x-redundant norm work disappears. All 40 window tiles are scaled
to bf16 z on DVE (4x mode), cast to fp8 in-flight by SWDGE bounce DMAs,
and pulled back through dma_gather(transpose=True) directly into the u16
pair-interleaved transposed layout. The similarity GEMM runs
DoubleRowSwInterleave with natural interleaved weights (psum rows come
back reversed — host relabels) and byte-strided moving APs: no
deinterleave pass. exp+rowsum on ACT in [128,1024] psum tiles; colsums
via ones-matmuls for the 3 symmetric exchange groups; positives from the
group-4 psum anti-diagonal. Host: rowsums + shifted colsums - e^2 - 2*pos.
"""

import numpy as np
import ml_dtypes

try:
    import concourse.bass as bass
except ImportError:
    import sys

    for _p in ("/opt/trn_rl_repo", "/root/.axon_site/_ro/trn_rl_repo"):
        if _p not in sys.path:
            sys.path.append(_p)
    import concourse.bass as bass

import concourse.mybir as mybir
import concourse.tile as tile
from concourse import bacc
from concourse.bass_utils import run_bass_kernel_spmd

F32 = mybir.dt.float32
BF16 = mybir.dt.bfloat16
FP8 = mybir.dt.float8e4
U16 = mybir.dt.uint16
I16 = mybir.dt.int16
AF = mybir.ActivationFunctionType
ALU = mybir.AluOpType
MM = mybir.MatmulPerfMode

B = 4096
D = 1024
R = 2 * B
N_CORES = 8
LOCAL = R // N_CORES          # 1024 rows per core
NG = 5                        # column groups per core (symmetry)
W = NG * LOCAL                # 5120 window rows/columns
KK = 4                        # u16 pair chunks (128 partitions each)
CS_G = (1, 2, 3)              # groups recovered via colsum exchange
INV_TEMP = 2.0
E2 = float(np.exp(INV_TEMP))
S = 16.0                      # fp8 pre-scale (avoids e4m3 subnormals)
EXP_SCALE = INV_TEMP / (S * S)


def build_nc():
    nc = bacc.Bacc("TRN2", target_bir_lowering=False, debug=False)

    xwin = nc.dram_tensor("xwin", [W, D], BF16, kind="ExternalInput")
    # z8 gather indices (identity into z8win; static, same on every core):
    # chunk h=j//512, i=j%512 -> slot [i%16 (+16), g, h*32 + i//16]
    idxs = nc.dram_tensor("idxs", [128, NG, LOCAL // 16], I16, kind="ExternalInput")
    # inv block-gather indices (per-core): window 64-row block -> global block
    invidx = nc.dram_tensor("invidx", [128, 8], I16, kind="ExternalInput")
    aeye = nc.dram_tensor("aeye", [128, 128], F32, kind="ExternalInput")

    out_rowsum = nc.dram_tensor("rowsum", [128, 8], F32, kind="ExternalOutput")
    out_pos = nc.dram_tensor("pos", [128, 8], F32, kind="ExternalOutput")
    out_colsum = nc.dram_tensor("colsum", [1, 3072], F32, kind="ExternalOutput")

    invloc = nc.dram_tensor("invloc", [LOCAL], F32)
    invfull = nc.dram_tensor("invfull", [R], F32, addr_space="Shared")
    z8win = nc.dram_tensor("z8win", [W, D], FP8)

    with tile.TileContext(nc) as tc:
        with (
            tc.tile_pool(name="xin", bufs=1) as xin_pool,
            tc.tile_pool(name="zb", bufs=2) as zb_pool,
            tc.tile_pool(name="zt", bufs=1) as zt_pool,
            tc.tile_pool(name="stats", bufs=1) as stats_pool,
            tc.tile_pool(name="scr", bufs=3) as scr_pool,
            tc.tile_pool(name="ej", bufs=3) as ej_pool,
            tc.tile_pool(name="ejcs", bufs=12) as ejcs_pool,
            tc.tile_pool(name="psum", bufs=3, space="PSUM") as psum_pool,
            tc.tile_pool(name="pscs", bufs=2, space="PSUM") as pscs_pool,
        ):
            # ---------- static small inputs ----------
            idx_sb = stats_pool.tile([128, NG, LOCAL // 16], I16, name="idx_sb")
            nc.sync.dma_start(out=idx_sb[:], in_=idxs[:, :, :])
            iidx_sb = stats_pool.tile([128, 8], I16, name="iidx_sb")
            nc.sync.dma_start(out=iidx_sb[:], in_=invidx[:, :])
            ae = stats_pool.tile([128, 128], F32, name="ae")
            nc.sync.dma_start(out=ae[:], in_=aeye[:, :])
            ones = stats_pool.tile([128, 1], BF16, name="ones")
            nc.vector.memset(ones[:], 1.0)

            ssq = stats_pool.tile([128, 8], F32, name="ssq")
            inv = stats_pool.tile([128, 8], F32, name="inv")
            rs = stats_pool.tile([128, 8 * NG], F32, name="rs")
            pos = stats_pool.tile([128, 8], F32, name="pos")
            colsum_acc = stats_pool.tile([1, 3072], F32, name="colsum_acc")

            # ---------- phase N: window loads + LOCAL norms ----------
            xg = []  # 5 group tiles [128, 8, 1024]
            for g in range(NG):
                xt = xin_pool.tile([128, 8, D], BF16, name=f"xg{g}")
                xg.append(xt)
                for h in range(2):
                    r0 = g * LOCAL + h * 512
                    nc.sync.dma_start(
                        out=xt[:, h * 4 : (h + 1) * 4, :],
                        in_=xwin[r0 : r0 + 512, :].rearrange("(t p) d -> p t d", p=128),
                    )
            # local norms: tiles 0-3 on DVE, 4-7 on ACT (Square+accum, same
            # activation table as Ln/Exp)
            for t in range(4):
                scr = scr_pool.tile([128, D], BF16, tag="nscr", name=f"nscr{t}")
                nc.vector.tensor_mul(scr[:], xg[0][:, t, :], xg[0][:, t, :])
                nc.vector.tensor_reduce(
                    out=ssq[:, t : t + 1], in_=scr[:],
                    axis=mybir.AxisListType.X, op=ALU.add,
                )
            for t in range(4, 8):
                scr = scr_pool.tile([128, D], BF16, tag="nscr", name=f"nscr{t}")
                nc.scalar.activation(
                    out=scr[:], in_=xg[0][:, t, :], func=AF.Square,
                    accum_out=ssq[:, t : t + 1],
                )
            # inv = S / sqrt(ssq) = exp(-0.5*ln(ssq) + ln(S))
            lns = stats_pool.tile([128, 8], F32, name="lns")
            lnS = stats_pool.tile([128, 1], F32, name="lnS")
            nc.vector.memset(lnS[:], float(np.log(S)))
            nc.scalar.activation(out=lns[:], in_=ssq[:], func=AF.Ln)
            nc.scalar.activation(
                out=inv[:], in_=lns[:], func=AF.Exp, bias=lnS[:], scale=-0.5
            )
            nc.sync.dma_start(
                out=invloc.rearrange("(t p) -> p t", p=128), in_=inv[:, :]
            )

            invw = stats_pool.tile([128, W // 128], F32, name="invw")
            ztus = {}

            def emit_scale_cast(g):
                zbg = zb_pool.tile([128, 8, D], BF16, tag="zbg", name=f"zbg{g}")
                for t in range(8):
                    w = g * 8 + t
                    sc = inv[:, t : t + 1] if g == 0 else invw[:, w : w + 1]
                    nc.vector.tensor_scalar_mul(zbg[:, t, :], xg[g][:, t, :], sc)
                for h in range(2):
                    nc.gpsimd.dma_start(
                        out=z8win[
                            g * LOCAL + h * 512 : g * LOCAL + (h + 1) * 512, :
                        ].rearrange("(t p) d -> p t d", p=128),
                        in_=zbg[:, h * 4 : (h + 1) * 4, :],
                    )

            def emit_zgather(g):
                for h in range(2):
                    ztu_gh = zt_pool.tile([128, KK, 512], U16, name=f"ztu{g}_{h}")
                    ztus[(g, h)] = ztu_gh
                    nc.gpsimd.dma_gather(
                        out_ap=ztu_gh[:],
                        in_ap=z8win.bitcast(U16)[:, :],
                        idxs_ap=idx_sb[:, g, h * 32 : (h + 1) * 32],
                        num_idxs=512, num_idxs_reg=512,
                        elem_size=D // 2, transpose=True,
                    )

            # ---------- g0 fast path: scale/cast, then AG, then g0 gather ----------
            emit_scale_cast(0)
            nc.gpsimd.collective_compute(
                "AllGather", ALU.bypass,
                ins=[invloc[:]], outs=[invfull[:]],
                replica_groups=[list(range(N_CORES))],
            )
            emit_zgather(0)
            # inv block-gather (64 f32 = 256B blocks) + redistribution
            pre = stats_pool.tile([128, 128], F32, name="pre")
            nc.vector.memset(pre[:], 1.0)
            nc.gpsimd.dma_gather(
                out_ap=pre[:, 0:64].unsqueeze(1),
                in_ap=invfull.rearrange("(r o) -> r o", o=64),
                idxs_ap=iidx_sb[:],
                num_idxs=80, num_idxs_reg=80,
                elem_size=64, transpose=False,
            )
            preT = stats_pool.tile([128, 128], F32, name="preT")
            nc.vector.transpose(preT[:], pre[:])
            # invw[p, w] = inv(window row w*128+p):
            #   p<64:  preT[p, 2w]      p>=64: preT[p-64, 2w+1]
            nc.vector.tensor_copy(invw[0:64, :], preT[0:64, 0 : 2 * (W // 128) : 2])
            nc.vector.tensor_copy(invw[64:128, :], preT[0:64, 1 : 2 * (W // 128) : 2])

            # ---------- phase G: GEMM + exp (+colsum, +pos) ----------
            # SwInterleave: psum partition p = local row m*128 + (127-p)
            cs_tiles = {}

            def emit_colsum(g):
                css = cs_tiles[g]
                for m in range(8):
                    ej = cs_tiles[(g, m)]
                    for cb in range(2):
                        nc.tensor.matmul(
                            css[cb][:], ones[:],
                            ej[:, cb * 512 : (cb + 1) * 512],
                            start=(m == 0), stop=(m == 7),
                        )
                for cb in range(2):
                    off = (g - 1) * 1024 + cb * 512
                    nc.vector.tensor_copy(colsum_acc[:, off : off + 512], css[cb][:])

            for g in range(NG):
                if g >= 1:
                    emit_scale_cast(g)
                    emit_zgather(g)
                if g - 1 in CS_G:
                    emit_colsum(g - 1)
                if g in CS_G:
                    cs_tiles[g] = [
                        pscs_pool.tile([1, 512], F32, tag="cs", name=f"cs{g}_{cb}")
                        for cb in range(2)
                    ]
                for m in range(8):
                    ps = psum_pool.tile([128, 1024], F32, tag="ps", name=f"ps{g}_{m}")
                    for kk in range(KK):
                        w_ap = ztus[(0, m // 4)][
                            :, kk, (m % 4) * 128 : (m % 4 + 1) * 128
                        ].bitcast(FP8)
                        for cb in range(2):
                            rhs = (
                                ztus[(g, cb)][:, kk, :]
                                .bitcast(FP8)
                                .rearrange("p (n two) -> p two n", two=2)
                            )
                            nc.tensor.matmul(
                                ps[:, cb * 512 : (cb + 1) * 512],
                                w_ap, rhs,
                                start=(kk == 0), stop=(kk == KK - 1),
                                perf_mode=MM.DoubleRowSwInterleave,
                            )
                    if g in CS_G:
                        ej = ejcs_pool.tile(
                            [128, 1024], BF16, tag="ejcs", name=f"ej{g}_{m}"
                        )
                        cs_tiles[(g, m)] = ej
                    else:
                        ej = ej_pool.tile([128, 1024], BF16, tag="ej", name=f"ej{g}_{m}")
                    nc.scalar.activation(
                        out=ej[:], in_=ps[:], func=AF.Exp, bias=0.0, scale=EXP_SCALE,
                        accum_out=rs[:, (m * NG + g) : (m * NG + g) + 1],
                    )
                    if g == 4:
                        pscr = scr_pool.tile([128, 128], F32, tag="pscr", name=f"pscr{m}")
                        nc.vector.tensor_tensor(
                            out=pscr[:], in0=ps[:, m * 128 : (m + 1) * 128],
                            in1=ae[:], op=ALU.mult,
                        )
                        nc.vector.tensor_reduce(
                            out=pos[:, m : m + 1], in_=pscr[:],
                            axis=mybir.AxisListType.X, op=ALU.add,
                        )
            emit_colsum(NG - 2)

            # ---------- outputs ----------
            rowsum = stats_pool.tile([128, 8], F32, name="rowsum")
            nc.vector.tensor_reduce(
                out=rowsum[:],
                in_=rs[:, :].rearrange("p (m g) -> p m g", g=NG),
                axis=mybir.AxisListType.X, op=ALU.add,
            )
            nc.sync.dma_start(out=out_rowsum[:, :], in_=rowsum[:])
            nc.sync.dma_start(out=out_pos[:, :], in_=pos[:])
            nc.sync.dma_start(out=out_colsum[:, :], in_=colsum_acc[:])

    nc.compile()
    return nc


_NC = None


def _get_nc():
    global _NC
    if _NC is None:
        _NC = build_nc()
    return _NC


_Z8IDX = np.zeros((128, NG, LOCAL // 16), np.int16)
for _g in range(NG):
    for _j in range(LOCAL):
        _h, _i = divmod(_j, 512)
        _p, _c = _i % 16, _h * 32 + _i // 16
        _Z8IDX[_p, _g, _c] = _g * LOCAL + _j
        _Z8IDX[16 + _p, _g, _c] = _g * LOCAL + _j

_AEYE = np.eye(128, dtype=np.float32)[:, ::-1].copy()


def _make_invidx(core):
    idx = np.zeros((128, 8), np.int16)
    for b in range(W // 64):
        blk = (core * (LOCAL // 64) + b) % (R // 64)
        idx[b % 16, b // 16] = blk
        idx[16 + b % 16, b // 16] = blk
    return idx


def make_in_maps(emb_i, emb_j):
    reps = np.concatenate(
        [np.asarray(emb_i, np.float32), np.asarray(emb_j, np.float32)], axis=0
    ).astype(ml_dtypes.bfloat16)
    maps = []
    for c in range(N_CORES):
        win = np.roll(reps, -c * LOCAL, axis=0)[:W]
        maps.append({
            "xwin": np.ascontiguousarray(win),
            "idxs": _Z8IDX,
            "invidx": _make_invidx(c),
            "aeye": _AEYE,
        })
    return maps


def run_spmd(in_maps, trace=False, **kwargs):
    return run_bass_kernel_spmd(
        _get_nc(), in_maps, core_ids=list(range(N_CORES)), trace=trace, **kwargs
    )


def combine(results):
    # SwInterleave row reversal: psum partition p of m-block = row m*128+(127-p)
    rowsum = np.zeros((N_CORES, LOCAL))
    pos = np.zeros((N_CORES, LOCAL))
    rev = 127 - np.arange(128)
    for c in range(N_CORES):
        rsum = results[c]["rowsum"].astype(np.float64)  # [128 p, 8 m]
        ps = results[c]["pos"].astype(np.float64)
        for m in range(8):
            rowsum[c, m * 128 + rev] = rsum[:, m]
            pos[c, m * 128 + rev] = ps[:, m]
    colsum = np.stack(
        [r["colsum"].astype(np.float64).reshape(3, 1024) for r in results]
    )
    denom = rowsum.copy()
    for b in range(N_CORES):
        for gp in CS_G:
            denom[b] += colsum[(b - gp) % N_CORES][gp - 1]
    denom -= E2
    loss_rows = np.log(denom) - (INV_TEMP / (S * S)) * pos
    return float(loss_rows.sum() / R)


def kernel(emb_i, emb_j):
    res = run_spmd(make_in_maps(emb_i, emb_j))
    return np.array(combine(res.results), dtype=np.float32)


# revision 17
# speedup vs baseline: 1.6619x; 1.1361x over previous
"""NT-Xent contrastive loss on 8 trn2 cores — inv-exchange architecture.

Each core receives the full rolled 5120-row window in bf16 (host-sharded)
but computes norms ONLY for its local 1024 rows (DVE square+reduce, ACT
Ln/Exp rsqrt with the fp8 pre-scale folded in). The tiny per-core inv
vectors are AllGathered (32KB — cheap) and gathered back per-window-row,
so the 5x-redundant norm work disappears. All 40 window tiles are scaled
to bf16 z on DVE (4x mode), cast to fp8 in-flight by SWDGE bounce DMAs,
and pulled back through dma_gather(transpose=True) directly into the u16
pair-interleaved transposed layout. The similarity GEMM runs
DoubleRowSwInterleave with natural interleaved weights (psum rows come
back reversed — host relabels) and byte-strided moving APs: no
deinterleave pass. exp+rowsum on ACT in [128,1024] psum tiles; colsums
via ones-matmuls for the 3 symmetric exchange groups; positives from the
group-4 psum anti-diagonal. Host: rowsums + shifted colsums - e^2 - 2*pos.
"""

import numpy as np
import ml_dtypes

try:
    import concourse.bass as bass
except ImportError:
    import sys

    for _p in ("/opt/trn_rl_repo", "/root/.axon_site/_ro/trn_rl_repo"):
        if _p not in sys.path:
            sys.path.append(_p)
    import concourse.bass as bass

import concourse.mybir as mybir
import concourse.tile as tile
from concourse import bacc
from concourse.bass_utils import run_bass_kernel_spmd

F32 = mybir.dt.float32
BF16 = mybir.dt.bfloat16
FP8 = mybir.dt.float8e4
U16 = mybir.dt.uint16
I16 = mybir.dt.int16
AF = mybir.ActivationFunctionType
ALU = mybir.AluOpType
MM = mybir.MatmulPerfMode

B = 4096
D = 1024
R = 2 * B
N_CORES = 8
LOCAL = R // N_CORES          # 1024 rows per core
NG = 5                        # column groups per core (symmetry)
W = NG * LOCAL                # 5120 window rows/columns
KK = 4                        # u16 pair chunks (128 partitions each)
CS_G = (1, 2, 3)              # groups recovered via colsum exchange
INV_TEMP = 2.0
E2 = float(np.exp(INV_TEMP))
S = 16.0                      # fp8 pre-scale (avoids e4m3 subnormals)
EXP_SCALE = INV_TEMP / (S * S)


def build_nc():
    nc = bacc.Bacc("TRN2", target_bir_lowering=False, debug=False)

    xwin = nc.dram_tensor("xwin", [W, D], BF16, kind="ExternalInput")
    # z8 gather indices (identity into z8win; static, same on every core):
    # chunk h=j//512, i=j%512 -> slot [i%16 (+16), g, h*32 + i//16]
    idxs = nc.dram_tensor("idxs", [128, NG, LOCAL // 16], I16, kind="ExternalInput")
    # inv block-gather indices (per-core): window 64-row block -> global block
    invidx = nc.dram_tensor("invidx", [128, 8], I16, kind="ExternalInput")
    aeye = nc.dram_tensor("aeye", [128, 128], F32, kind="ExternalInput")

    out_rowsum = nc.dram_tensor("rowsum", [128, 8], F32, kind="ExternalOutput")
    out_pos = nc.dram_tensor("pos", [128, 8], F32, kind="ExternalOutput")
    out_colsum = nc.dram_tensor("colsum", [1, 3072], F32, kind="ExternalOutput")

    invloc = nc.dram_tensor("invloc", [LOCAL], F32)
    invfull = nc.dram_tensor("invfull", [R], F32, addr_space="Shared")
    z8win = nc.dram_tensor("z8win", [W, D], FP8)

    with tile.TileContext(nc) as tc:
        with (
            tc.tile_pool(name="xin", bufs=4) as xin_pool,
            tc.tile_pool(name="zb", bufs=3) as zb_pool,
            tc.tile_pool(name="zt", bufs=1) as zt_pool,
            tc.tile_pool(name="stats", bufs=1) as stats_pool,
            tc.tile_pool(name="scr", bufs=2) as scr_pool,
            tc.tile_pool(name="ej", bufs=2) as ej_pool,
            tc.tile_pool(name="ejcs", bufs=12) as ejcs_pool,
            tc.tile_pool(name="psum", bufs=3, space="PSUM") as psum_pool,
            tc.tile_pool(name="pscs", bufs=2, space="PSUM") as pscs_pool,
        ):
            # ---------- static small inputs ----------
            idx_sb = stats_pool.tile([128, NG, LOCAL // 16], I16, name="idx_sb")
            nc.scalar.dma_start(out=idx_sb[:], in_=idxs[:, :, :])
            iidx_sb = stats_pool.tile([128, 8], I16, name="iidx_sb")
            nc.scalar.dma_start(out=iidx_sb[:], in_=invidx[:, :])
            ae = stats_pool.tile([128, 128], F32, name="ae")
            nc.scalar.dma_start(out=ae[:], in_=aeye[:, :])
            ones = stats_pool.tile([128, 1], BF16, name="ones")
            nc.vector.memset(ones[:], 1.0)

            ssq = stats_pool.tile([128, 8], F32, name="ssq")
            inv = stats_pool.tile([128, 8], F32, name="inv")
            rs = stats_pool.tile([128, 8 * NG], F32, name="rs")
            pos = stats_pool.tile([128, 8], F32, name="pos")
            colsum_acc = stats_pool.tile([1, 3072], F32, name="colsum_acc")

            # ---------- loads: g0+g1 first, then invloc write, then g2-4 ----
            # (the SP queue blocks at the invloc write's data dep, so the tiny
            # invloc DMA grabs the shared DMA engines before the bulk loads)
            xg = []
            for g in range(NG):
                xg.append(xin_pool.tile([128, 8, D], BF16, tag="xg", name=f"xg{g}"))

            def emit_load(g):
                for h in range(2):
                    r0 = g * LOCAL + h * 512
                    nc.sync.dma_start(
                        out=xg[g][:, h * 4 : (h + 1) * 4, :],
                        in_=xwin[r0 : r0 + 512, :].rearrange("(t p) d -> p t d", p=128),
                    )

            emit_load(0)
            emit_load(1)

            # ---------- local norms: tiles 0-3 on DVE, 4-7 on ACT ----------
            for t in range(4):
                scr = scr_pool.tile([128, D], BF16, tag="nscr", name=f"nscr{t}")
                nc.vector.tensor_mul(scr[:], xg[0][:, t, :], xg[0][:, t, :])
                nc.vector.tensor_reduce(
                    out=ssq[:, t : t + 1], in_=scr[:],
                    axis=mybir.AxisListType.X, op=ALU.add,
                )
            for t in range(4, 8):
                scr = scr_pool.tile([128, D], BF16, tag="nscr", name=f"nscr{t}")
                nc.scalar.activation(
                    out=scr[:], in_=xg[0][:, t, :], func=AF.Square,
                    accum_out=ssq[:, t : t + 1],
                )
            # inv = S / sqrt(ssq) = exp(-0.5*ln(ssq) + ln(S))
            lns = stats_pool.tile([128, 8], F32, name="lns")
            lnS = stats_pool.tile([128, 1], F32, name="lnS")
            nc.vector.memset(lnS[:], float(np.log(S)))
            nc.scalar.activation(out=lns[:], in_=ssq[:], func=AF.Ln)
            nc.scalar.activation(
                out=inv[:], in_=lns[:], func=AF.Exp, bias=lnS[:], scale=-0.5
            )
            nc.sync.dma_start(
                out=invloc.rearrange("(t p) -> p t", p=128), in_=inv[:, :]
            )
            for g in range(2, NG):
                emit_load(g)

            invw = stats_pool.tile([128, W // 128], F32, name="invw")
            ztus = {}

            def emit_scale(g):
                zbg = zb_pool.tile([128, 8, D], BF16, tag="zbg", name=f"zbg{g}")
                for t in range(8):
                    w = g * 8 + t
                    sc = inv[:, t : t + 1] if g == 0 else invw[:, w : w + 1]
                    nc.vector.tensor_scalar_mul(zbg[:, t, :], xg[g][:, t, :], sc)
                return zbg

            def emit_castw(g, zbg):
                for h in range(2):
                    nc.gpsimd.dma_start(
                        out=z8win[
                            g * LOCAL + h * 512 : g * LOCAL + (h + 1) * 512, :
                        ].rearrange("(t p) d -> p t d", p=128),
                        in_=zbg[:, h * 4 : (h + 1) * 4, :],
                    )

            def emit_zgather(g):
                for h in range(2):
                    ztu_gh = zt_pool.tile([128, KK, 512], U16, name=f"ztu{g}_{h}")
                    ztus[(g, h)] = ztu_gh
                    nc.gpsimd.dma_gather(
                        out_ap=ztu_gh[:],
                        in_ap=z8win.bitcast(U16)[:, :],
                        idxs_ap=idx_sb[:, g, h * 32 : (h + 1) * 32],
                        num_idxs=512, num_idxs_reg=512,
                        elem_size=D // 2, transpose=True,
                    )

            # ---------- g0 fast path + inv exchange ----------
            zbg0 = emit_scale(0)
            emit_castw(0, zbg0)
            nc.gpsimd.collective_compute(
                "AllGather", ALU.bypass,
                ins=[invloc[:]], outs=[invfull[:]],
                replica_groups=[list(range(N_CORES))],
            )
            emit_zgather(0)
            pre = stats_pool.tile([128, 128], F32, name="pre")
            nc.vector.memset(pre[:], 1.0)
            nc.gpsimd.dma_gather(
                out_ap=pre[:, 0:64].unsqueeze(1),
                in_ap=invfull.rearrange("(r o) -> r o", o=64),
                idxs_ap=iidx_sb[:],
                num_idxs=80, num_idxs_reg=80,
                elem_size=64, transpose=False,
            )
            preT = stats_pool.tile([128, 128], F32, name="preT")
            nc.vector.transpose(preT[:], pre[:])
            # invw[p, w] = inv(window row w*128+p):
            #   p<64: preT[p, 2w]   p>=64: preT[p-64, 2w+1]
            nc.vector.tensor_copy(invw[0:64, :], preT[0:64, 0 : 2 * (W // 128) : 2])
            nc.vector.tensor_copy(invw[64:128, :], preT[0:64, 1 : 2 * (W // 128) : 2])

            # ---------- phase G ----------
            # SwInterleave: psum partition p = local row m*128 + (127-p)
            cs_ej = {}

            def emit_gemm(g):
                for m in range(8):
                    ps = psum_pool.tile([128, 1024], F32, tag="ps", name=f"ps{g}_{m}")
                    for kk in range(KK):
                        w_ap = ztus[(0, m // 4)][
                            :, kk, (m % 4) * 128 : (m % 4 + 1) * 128
                        ].bitcast(FP8)
                        for cb in range(2):
                            rhs = (
                                ztus[(g, cb)][:, kk, :]
                                .bitcast(FP8)
                                .rearrange("p (n two) -> p two n", two=2)
                            )
                            nc.tensor.matmul(
                                ps[:, cb * 512 : (cb + 1) * 512],
                                w_ap, rhs,
                                start=(kk == 0), stop=(kk == KK - 1),
                                perf_mode=MM.DoubleRowSwInterleave,
                            )
                    if g in CS_G:
                        ej = ejcs_pool.tile(
                            [128, 1024], BF16, tag="ejcs", name=f"ej{g}_{m}"
                        )
                        cs_ej[(g, m)] = ej
                    else:
                        ej = ej_pool.tile([128, 1024], BF16, tag="ej", name=f"ej{g}_{m}")
                    nc.scalar.activation(
                        out=ej[:], in_=ps[:], func=AF.Exp, bias=0.0, scale=EXP_SCALE,
                        accum_out=rs[:, (m * NG + g) : (m * NG + g) + 1],
                    )
                    if g == 4:
                        pscr = scr_pool.tile([128, 128], F32, tag="pscr", name=f"pscr{m}")
                        nc.vector.tensor_tensor(
                            out=pscr[:], in0=ps[:, m * 128 : (m + 1) * 128],
                            in1=ae[:], op=ALU.mult,
                        )
                        nc.vector.tensor_reduce(
                            out=pos[:, m : m + 1], in_=pscr[:],
                            axis=mybir.AxisListType.X, op=ALU.add,
                        )

            def emit_colsum(g):
                css = [
                    pscs_pool.tile([1, 512], F32, tag="cs", name=f"cs{g}_{cb}")
                    for cb in range(2)
                ]
                for m in range(8):
                    ej = cs_ej[(g, m)]
                    for cb in range(2):
                        nc.tensor.matmul(
                            css[cb][:], ones[:],
                            ej[:, cb * 512 : (cb + 1) * 512],
                            start=(m == 0), stop=(m == 7),
                        )
                for cb in range(2):
                    off = (g - 1) * 1024 + cb * 512
                    nc.vector.tensor_copy(colsum_acc[:, off : off + 512], css[cb][:])

            emit_gemm(0)
            for g in range(1, NG):
                zbg = emit_scale(g)
                emit_castw(g, zbg)
                emit_zgather(g)
                emit_gemm(g)
                if g - 1 in CS_G:
                    emit_colsum(g - 1)
            emit_colsum(NG - 2)

            # ---------- outputs ----------
            rowsum = stats_pool.tile([128, 8], F32, name="rowsum")
            nc.vector.tensor_reduce(
                out=rowsum[:],
                in_=rs[:, :].rearrange("p (m g) -> p m g", g=NG),
                axis=mybir.AxisListType.X, op=ALU.add,
            )
            nc.sync.dma_start(out=out_rowsum[:, :], in_=rowsum[:])
            nc.sync.dma_start(out=out_pos[:, :], in_=pos[:])
            nc.sync.dma_start(out=out_colsum[:, :], in_=colsum_acc[:])

    nc.compile()
    return nc


_NC = None


def _get_nc():
    global _NC
    if _NC is None:
        _NC = build_nc()
    return _NC


_Z8IDX = np.zeros((128, NG, LOCAL // 16), np.int16)
for _g in range(NG):
    for _j in range(LOCAL):
        _h, _i = divmod(_j, 512)
        _p, _c = _i % 16, _h * 32 + _i // 16
        _Z8IDX[_p, _g, _c] = _g * LOCAL + _j
        _Z8IDX[16 + _p, _g, _c] = _g * LOCAL + _j

_AEYE = np.eye(128, dtype=np.float32)[:, ::-1].copy()


def _make_invidx(core):
    idx = np.zeros((128, 8), np.int16)
    for b in range(W // 64):
        blk = (core * (LOCAL // 64) + b) % (R // 64)
        idx[b % 16, b // 16] = blk
        idx[16 + b % 16, b // 16] = blk
    return idx


def make_in_maps(emb_i, emb_j):
    reps = np.concatenate(
        [np.asarray(emb_i, np.float32), np.asarray(emb_j, np.float32)], axis=0
    ).astype(ml_dtypes.bfloat16)
    maps = []
    for c in range(N_CORES):
        win = np.roll(reps, -c * LOCAL, axis=0)[:W]
        maps.append({
            "xwin": np.ascontiguousarray(win),
            "idxs": _Z8IDX,
            "invidx": _make_invidx(c),
            "aeye": _AEYE,
        })
    return maps


def run_spmd(in_maps, trace=False, **kwargs):
    return run_bass_kernel_spmd(
        _get_nc(), in_maps, core_ids=list(range(N_CORES)), trace=trace, **kwargs
    )


def combine(results):
    # SwInterleave row reversal: psum partition p of m-block = row m*128+(127-p)
    rowsum = np.zeros((N_CORES, LOCAL))
    pos = np.zeros((N_CORES, LOCAL))
    rev = 127 - np.arange(128)
    for c in range(N_CORES):
        rsum = results[c]["rowsum"].astype(np.float64)  # [128 p, 8 m]
        ps = results[c]["pos"].astype(np.float64)
        for m in range(8):
            rowsum[c, m * 128 + rev] = rsum[:, m]
            pos[c, m * 128 + rev] = ps[:, m]
    colsum = np.stack(
        [r["colsum"].astype(np.float64).reshape(3, 1024) for r in results]
    )
    denom = rowsum.copy()
    for b in range(N_CORES):
        for gp in CS_G:
            denom[b] += colsum[(b - gp) % N_CORES][gp - 1]
    denom -= E2
    loss_rows = np.log(denom) - (INV_TEMP / (S * S)) * pos
    return float(loss_rows.sum() / R)


def kernel(emb_i, emb_j):
    res = run_spmd(make_in_maps(emb_i, emb_j))
    return np.array(combine(res.results), dtype=np.float32)
